# revision 34
# baseline (speedup 1.0000x reference)
import sys

import numpy as np

sys.path.insert(0, "/opt/trn_rl_repo")

TRACE = False
LAST = {}
_cache = {}

SPARSITY = 0.5


def _tf32(a):
    b = np.ascontiguousarray(np.asarray(a, np.float32))
    u = b.view(np.uint32).copy()
    u += np.uint32(0x0FFF) + ((u >> np.uint32(13)) & np.uint32(1))
    u &= np.uint32(0xFFFFE000)
    return u.view(np.float32)


def _masked(w, s):
    sa = np.abs(np.asarray(s, np.float32)).ravel()
    j = int((1.0 - SPARSITY) * sa.size)
    thr = np.partition(sa, j)[j]
    m = (np.abs(np.asarray(s, np.float32)) >= thr).astype(np.float32)
    return (np.asarray(w, np.float32) * m).astype(np.float32)


def _prep(inputs):
    w1m = _masked(inputs["w1"], inputs["s1"])  # [128,3,3,3]
    w2m = _masked(inputs["w2"], inputs["s2"])  # [256,128,3,3]
    w3m = _masked(inputs["w3"], inputs["s3"])  # [512,256,3,3]
    fw1m = _masked(inputs["fw1"], inputs["fs1"])  # [1024,512]
    fw2m = _masked(inputs["fw2"], inputs["fs2"])  # [10,1024]

    c = np.ascontiguousarray
    # conv1 as single K=27 matmul: lhsT [27(ch,ky,kx), 128]
    w1t = c(w1m.transpose(1, 2, 3, 0).reshape(27, 128))
    w2t = c(w2m.transpose(1, 2, 3, 0).reshape(128, 9 * 256))
    # mt-major: [k2, mt, kt*1152 + g*128 + o]
    w3t = c(
        w3m.reshape(4, 128, 2, 128, 3, 3)
        .transpose(3, 0, 2, 4, 5, 1)
        .reshape(128, 4, 2304)
    )
    # global-avg-pool 1/256 folded into fw1
    fw1t = c((fw1m.T.reshape(4, 128, 1024).transpose(1, 0, 2) / 256.0).astype(np.float32))
    fw2t = c(fw2m.T.reshape(8, 128, 10).transpose(1, 0, 2))

    weights = {
        "w1t": _tf32(w1t),
        "w2t": _tf32(w2t),
        "w3t": _tf32(w3t),
        "fw1t": fw1t,
        "fw2t": fw2t,
        "b1": c(np.asarray(inputs["b1"], np.float32).reshape(128, 1)),
        "b2": c(np.asarray(inputs["b2"], np.float32).reshape(2, 128).T),
        "b3": c(np.asarray(inputs["b3"], np.float32).reshape(4, 128).T),
        "fb1": c(np.asarray(inputs["fb1"], np.float32).reshape(1, 1024)),
        "fb2": c(np.asarray(inputs["fb2"], np.float32).reshape(10, 1)),
    }
    xpad = np.zeros((64, 3, 66, 66), np.float32)
    xpad[:, :, 1:65, 1:65] = _tf32(inputs["x"])
    # im2col over (ch,ky,kx): x27[i, ch*9+ky*3+kx] = xpad[i, ch, ky:ky+64, kx:kx+64]
    x27 = np.empty((64, 27, 64, 64), np.float32)
    for ch in range(3):
        for ky in range(3):
            for kx in range(3):
                x27[:, ch * 9 + ky * 3 + kx] = xpad[:, ch, ky : ky + 64, kx : kx + 64]
    return x27, weights


def _build():
    import concourse.bacc as bacc
    import concourse.mybir as mybir
    import concourse.tile as tile

    FP = mybir.dt.float32
    FR = mybir.dt.float32r
    RELU = mybir.ActivationFunctionType.Relu
    ADD = mybir.AluOpType.add
    MAX = mybir.AluOpType.max

    nc = bacc.Bacc("TRN2", target_bir_lowering=False, debug=False)

    xpad_d = nc.dram_tensor("xpad", [8, 27, 64, 64], FR, kind="ExternalInput")
    w1t_d = nc.dram_tensor("w1t", [27, 128], FR, kind="ExternalInput")
    w2t_d = nc.dram_tensor("w2t", [128, 2304], FR, kind="ExternalInput")
    w3t_d = nc.dram_tensor("w3t", [128, 4, 2304], FR, kind="ExternalInput")
    fw1t_d = nc.dram_tensor("fw1t", [128, 4, 1024], FP, kind="ExternalInput")
    fw2t_d = nc.dram_tensor("fw2t", [128, 8, 10], FP, kind="ExternalInput")
    b1_d = nc.dram_tensor("b1", [128, 1], FP, kind="ExternalInput")
    b2_d = nc.dram_tensor("b2", [128, 2], FP, kind="ExternalInput")
    b3_d = nc.dram_tensor("b3", [128, 4], FP, kind="ExternalInput")
    fb1_d = nc.dram_tensor("fb1", [1, 1024], FP, kind="ExternalInput")
    fb2_d = nc.dram_tensor("fb2", [10, 1], FP, kind="ExternalInput")
    outT_d = nc.dram_tensor("outT", [10, 8], FP, kind="ExternalOutput")

    with tile.TileContext(nc) as tc:
        with tc.tile_pool(name="consts", bufs=1) as consts, \
             tc.tile_pool(name="xim_p", bufs=2) as xim_p, \
             tc.tile_pool(name="act_p", bufs=1) as act_p, \
             tc.tile_pool(name="h3_p", bufs=2) as h3_p, \
             tc.tile_pool(name="ps1_p", bufs=4, space="PSUM") as ps1_p, \
             tc.tile_pool(name="ps2_p", bufs=2, space="PSUM") as ps2_p, \
             tc.tile_pool(name="ps3_p", bufs=2, space="PSUM") as ps3_p:

            xims = {}

            def load_xim_half(img, half):
                t = xim_p.tile([27, 32, 64], FR, name=f"ximh{half}")
                nc.sync.dma_start(
                    out=t[:, :, :], in_=xpad_d[img, :, 32 * half : 32 * half + 32, :]
                )
                xims[(img, half)] = t

            # DMA issue order = global transfer order. Tiny consts first, then
            # image halves interleaved with the weight streams they unblock.
            w1t = consts.tile([27, 128], FR)
            nc.sync.dma_start(out=w1t[:, :], in_=w1t_d[:, :])
            b1sb = consts.tile([128, 1], FP)
            nc.sync.dma_start(out=b1sb[:, :], in_=b1_d[:, :])
            load_xim_half(0, 0)
            w2t = consts.tile([128, 2304], FR)
            nc.sync.dma_start(out=w2t[:, :], in_=w2t_d[:, :])
            load_xim_half(0, 1)
            b2sb = consts.tile([128, 2], FP)
            nc.sync.dma_start(out=b2sb[:, :], in_=b2_d[:, :])
            load_xim_half(1, 0)
            load_xim_half(1, 1)
            w3sb = []
            for mt in range(4):
                t = consts.tile([128, 2304], FR, name=f"w3_mt{mt}")
                nc.sync.dma_start(out=t[:, :], in_=w3t_d[:, mt, :])
                w3sb.append(t)
            b3sb = consts.tile([128, 4], FP)
            nc.sync.dma_start(out=b3sb[:, :], in_=b3_d[:, :])

            # PE p-state warm-up: ramp runs on wall time since first dispatch,
            # so a burst of throwaway matmuls during the DMA wait gets the
            # engine to full clock before conv1 starts.
            warm = consts.tile([27, 8, 64], FR)
            nc.vector.memset(warm[:, :, :].bitcast(FP), 0.0)
            ones = consts.tile([1, 8], FP)
            nc.vector.memset(ones[:, :], 1.0)
            for _ in range(7):
                wps = ps1_p.tile([128, 8, 64], FP, name="ps_c1")
                nc.tensor.matmul(
                    out=wps[:, :, :], lhsT=w1t[:, :], rhs=warm[:, :, :],
                    start=True, stop=True,
                )

            h1pad_a = act_p.tile([128, 66, 66], FR)
            h1pad_b = act_p.tile([128, 66, 66], FR)
            h2pad_a = act_p.tile([128, 2, 2, 34, 34], FR)
            h2pad_b = act_p.tile([128, 2, 2, 34, 34], FR)
            hpool = act_p.tile([128, 4, 8], FP)
            z1T = act_p.tile([128, 8, 8], FP)
            y_sb = act_p.tile([10, 8], FP)

            # Border-only zeroing: interiors are fully overwritten every image,
            # borders stay zero for the kernel's lifetime.
            for h1 in (h1pad_a, h1pad_b):
                nc.vector.memset(h1[:, 0, :].bitcast(FP), 0.0)
                nc.vector.memset(h1[:, 65, :].bitcast(FP), 0.0)
                nc.vector.memset(h1[:, 1:65, 0].bitcast(FP), 0.0)
                nc.vector.memset(h1[:, 1:65, 65].bitcast(FP), 0.0)
            for h2 in (h2pad_a, h2pad_b):
                for m in range(2):
                    for i in range(2):
                        nc.vector.memset(h2[:, m, i, 0, :].bitcast(FP), 0.0)
                        nc.vector.memset(h2[:, m, i, 33, :].bitcast(FP), 0.0)
                        nc.vector.memset(h2[:, m, i, 1:33, 0].bitcast(FP), 0.0)
                        nc.vector.memset(h2[:, m, i, 1:33, 33].bitcast(FP), 0.0)

            h1pads = [h1pad_a, h1pad_b]
            h2pads = [h2pad_a, h2pad_b]

            def drain(eng_act, out, ps, bias):
                if eng_act:
                    nc.scalar.activation(out=out, in_=ps, func=RELU, bias=bias)
                else:
                    nc.vector.tensor_scalar(
                        out=out, in0=ps, scalar1=bias, scalar2=0.0,
                        op0=ADD, op1=MAX,
                    )

            def conv1_half(img, h1pad, half):
                xim = xims.pop((img, half))
                for nt in range(4):
                    ntg = 4 * half + nt
                    ps = ps1_p.tile([128, 8, 64], FP, name="ps_c1")
                    nc.tensor.matmul(
                        out=ps[:, :, :],
                        lhsT=w1t[:, :],
                        rhs=xim[:, 8 * nt : 8 * nt + 8, :],
                        start=True,
                        stop=True,
                    )
                    drain(
                        ntg % 2 == 0,
                        h1pad[:, 1 + 8 * ntg : 9 + 8 * ntg, 1:65],
                        ps[:, :, :],
                        b1sb[:, 0:1],
                    )

            def conv2_half(img, h1pad, h2pad, islot, nh):
                for m in range(2):
                    ps = ps2_p.tile([128, 16, 32], FP, name="ps_c2")
                    for g in range(9):
                        ky, kx = g // 3, g % 3
                        nc.tensor.matmul(
                            out=ps[:, :, :],
                            lhsT=w2t[:, 256 * g + 128 * m : 256 * g + 128 * m + 128],
                            rhs=h1pad[:, 32 * nh + ky : 32 * nh + ky + 32 : 2, kx : kx + 64 : 2],
                            start=(g == 0),
                            stop=(g == 8),
                        )
                    drain(
                        m == 0,
                        h2pad[:, m, islot, 1 + 16 * nh : 17 + 16 * nh, 1:33],
                        ps[:, :, :],
                        b2sb[:, m : m + 1],
                    )

            def conv3(pair, h2pad):
                for mt in range(4):
                    ps = ps3_p.tile([128, 2, 16, 16], FP, name="ps_c3")
                    n = 0
                    for kt in range(2):
                        for g in range(9):
                            ky, kx = g // 3, g % 3
                            nc.tensor.matmul(
                                out=ps[:, :, :, :],
                                lhsT=w3sb[mt][:, 1152 * kt + 128 * g : 1152 * kt + 128 * g + 128],
                                rhs=h2pad[:, kt, :, ky : ky + 32 : 2, kx : kx + 32 : 2],
                                start=(n == 0),
                                stop=(n == 17),
                            )
                            n += 1
                    h3 = h3_p.tile([128, 2, 16, 16], FP, name="h3scr")
                    nc.scalar.activation(
                        out=h3[:, 0, :, :],
                        in_=ps[:, 0, :, :],
                        func=RELU,
                        bias=b3sb[:, mt : mt + 1],
                        accum_out=hpool[:, mt, 2 * pair : 2 * pair + 1],
                    )
                    nc.vector.tensor_scalar(
                        out=h3[:, 1, :, :], in0=ps[:, 1, :, :],
                        scalar1=b3sb[:, mt : mt + 1], scalar2=0.0,
                        op0=ADD, op1=MAX,
                    )
                    nc.vector.tensor_reduce(
                        out=hpool[:, mt, 2 * pair + 1 : 2 * pair + 2],
                        in_=h3[:, 1, :, :],
                        axis=mybir.AxisListType.XY,
                        op=ADD,
                    )

            for pair in range(4):
                h2pad = h2pads[pair % 2]
                for i in range(2):
                    img = 2 * pair + i
                    h1pad = h1pads[img % 2]
                    if img == 0:
                        # xim halves still streaming in: old interleave
                        for half in range(2):
                            conv1_half(img, h1pad, half)
                            load_xim_half(img + 2, half)
                            conv2_half(img, h1pad, h2pad, i, half)
                    else:
                        # both conv1 halves first so their drains overlap the
                        # long conv2 matmul stretch instead of stalling it
                        for half in range(2):
                            conv1_half(img, h1pad, half)
                            if img + 2 < 8:
                                load_xim_half(img + 2, half)
                        for half in range(2):
                            conv2_half(img, h1pad, h2pad, i, half)
                conv3(pair, h2pad)
                if pair == 0:
                    fw1t = consts.tile([128, 4, 1024], FP)
                    for kt in range(4):
                        nc.sync.dma_start(out=fw1t[:, kt, :], in_=fw1t_d[:, kt, :])
                    fw2t = consts.tile([128, 8, 10], FP)
                    nc.sync.dma_start(out=fw2t[:, :, :], in_=fw2t_d[:, :, :])
                    fb1row = consts.tile([1, 1024], FP)
                    nc.sync.dma_start(out=fb1row[:, :], in_=fb1_d[:, :])
                    fb2sb = consts.tile([10, 1], FP)
                    nc.sync.dma_start(out=fb2sb[:, :], in_=fb2_d[:, :])

            # FC1: bias folded in as a K=1 matmul against a ones vector so all
            # eight m-groups accumulate into one PSUM tile and drain with a
            # single ACT (no per-group WAR serialization).
            psf = ps1_p.tile([128, 8, 8], FP, name="ps_c1")
            for m in range(8):
                for kt in range(4):
                    nc.tensor.matmul(
                        out=psf[:, m, :],
                        lhsT=fw1t[:, kt, 128 * m : 128 * m + 128],
                        rhs=hpool[:, kt, :],
                        start=(kt == 0),
                        stop=False,
                    )
                nc.tensor.matmul(
                    out=psf[:, m, :],
                    lhsT=fb1row[0:1, 128 * m : 128 * m + 128],
                    rhs=ones[0:1, :],
                    start=False,
                    stop=True,
                )
            nc.scalar.activation(out=z1T[:, :, :], in_=psf[:, :, :], func=RELU)

            psf2 = ps1_p.tile([128, 8], FP, name="ps_c1")
            for kt in range(8):
                nc.tensor.matmul(
                    out=psf2[0:10, :],
                    lhsT=fw2t[:, kt, :],
                    rhs=z1T[:, kt, :],
                    start=(kt == 0),
                    stop=(kt == 7),
                )
            nc.vector.tensor_scalar_add(y_sb[:, :], psf2[0:10, :], fb2sb[:, 0:1])
            nc.sync.dma_start(out=outT_d[:, :], in_=y_sb[:, :])

    nc.compile()
    return nc


def _get_nc():
    if "nc" not in _cache:
        _cache["nc"] = _build()
    return _cache["nc"]


def kernel(**inputs):
    from concourse import bass_utils

    nc = _get_nc()
    xpad, weights = _prep(inputs)
    in_maps = [
        dict(weights, xpad=np.ascontiguousarray(xpad[8 * c : 8 * c + 8]))
        for c in range(8)
    ]
    res = bass_utils.run_bass_kernel_spmd(
        nc, in_maps, core_ids=list(range(8)), trace=TRACE
    )
    LAST["exec_time_ns"] = getattr(res, "exec_time_ns", None)
    LAST["profile_json"] = getattr(res, "profile_json", None)
    LAST["instructions_and_trace"] = getattr(res, "instructions_and_trace", None)
    out = np.concatenate([r["outT"].T for r in res.results], axis=0)
    return np.ascontiguousarray(out.astype(np.float32))
